# revision 1
# baseline (speedup 1.0000x reference)
"""GRU seq2seq autoencoder (B=1024, T=512, C=32, H=256) on 8 trn2 NeuronCores.

Strategy: data-parallel over batch (128 rows/core, weights replicated).
Per-core layout is feature-major: hidden state h lives in SBUF as
[128 partitions, 2*128] where column-chunk k holds features 128k..128k+127
for all 128 batch rows. All matmuls are out[features, batch] =
W_chunk @ h (lhsT = W.T chunk stationary, rhs = h chunk streaming), so the
recurrence needs no transposes anywhere.

Bias folding: input-projection biases (+ r/z recurrent biases) ride on an
augmented ones-row of x through the Wih matmul; the n-gate recurrent bias
(inside the r* product) is added via a K=1 rank-1 matmul into the same PSUM
accumulation group. Gates then need only: one sigmoid over [128,512] (r|z),
2 DVE tensor ops, one tanh over [128,256], and 3 DVE tensor ops per step.

Matmul inputs and gate tiles are fp16 (PE streams 16-bit at 4x the fp32
rate; DVE tensor_tensor gets 2x mode); PSUM accumulation stays fp32.
"""

import os

import ml_dtypes
import numpy as np

import concourse.bacc as bacc
import concourse.mybir as mybir
import concourse.tile as tile
from concourse.bass_utils import run_bass_kernel_spmd

B, T, C, H = 1024, 512, 32, 256
NCORES = 8
BC = B // NCORES  # batch per core = 128
CA = C + 1  # augmented input rows (ones row carries biases)
XBLK = 32  # timesteps per x-stream DMA block
F32 = mybir.dt.float32
AF = mybir.ActivationFunctionType
OP = mybir.AluOpType

# Best measured config (A/B on hardware): split r/z sigmoid (shorter
# dependency chain), keep all gate tensor ops on the vector engine
# (GPSIMD offload loses to SBUF-port contention).
SPLIT_SIG = True
GP_OFFLOAD = False

MM_DT = mybir.dt.float16
NP_MM = ml_dtypes.float16 if hasattr(ml_dtypes, "float16") else np.float16
GATE_DT = MM_DT  # dtype of rz/n/t1/q/d/e/h tiles


def build(t_steps=T, reps=1):
    nblk = (t_steps + XBLK - 1) // XBLK
    assert t_steps % XBLK == 0 or t_steps < XBLK
    xblk = min(XBLK, t_steps)
    nc = bacc.Bacc("TRN2", num_devices=NCORES)

    xd = nc.dram_tensor("x_t", [nblk, CA, xblk * BC], MM_DT, kind="ExternalInput").ap()
    whh_e_d = nc.dram_tensor("whh_e", [128, 12 * 128], MM_DT, kind="ExternalInput").ap()
    whh_d_d = nc.dram_tensor("whh_d", [128, 12 * 128], MM_DT, kind="ExternalInput").ap()
    wih_e_d = nc.dram_tensor("wih_e", [CA, 768], MM_DT, kind="ExternalInput").ap()
    wih_d_d = nc.dram_tensor("wih_d", [CA, 768], MM_DT, kind="ExternalInput").ap()
    bhn_e_d = nc.dram_tensor("bhn_e", [1, 256], MM_DT, kind="ExternalInput").ap()
    bhn_d_d = nc.dram_tensor("bhn_d", [1, 256], MM_DT, kind="ExternalInput").ap()
    projT_d = nc.dram_tensor("projT", [128, 64], MM_DT, kind="ExternalInput").ap()
    projb_d = nc.dram_tensor("projb", [32, 1], F32, kind="ExternalInput").ap()
    yd = nc.dram_tensor("y_t", [t_steps, C, BC], F32, kind="ExternalOutput").ap()

    with tile.TileContext(nc) as tc:
        with (
            tc.tile_pool(name="const", bufs=1) as constp,
            tc.tile_pool(name="xp", bufs=2) as xp,
            tc.tile_pool(name="state", bufs=2) as statep,
            tc.tile_pool(name="work", bufs=2) as workp,
            tc.tile_pool(name="psum", bufs=2, space="PSUM") as psump,
        ):
            whh_e = constp.tile([128, 1536], MM_DT)
            nc.sync.dma_start(whh_e[:], whh_e_d[:])
            whh_d = constp.tile([128, 1536], MM_DT)
            nc.sync.dma_start(whh_d[:], whh_d_d[:])
            wih_e = constp.tile([CA, 768], MM_DT)
            nc.sync.dma_start(wih_e[:], wih_e_d[:])
            wih_d = constp.tile([CA, 768], MM_DT)
            nc.sync.dma_start(wih_d[:], wih_d_d[:])
            bhn_e = constp.tile([1, 256], MM_DT)
            nc.sync.dma_start(bhn_e[:], bhn_e_d[:])
            bhn_d = constp.tile([1, 256], MM_DT)
            nc.sync.dma_start(bhn_d[:], bhn_d_d[:])
            projT = constp.tile([128, 64], MM_DT)
            nc.sync.dma_start(projT[:], projT_d[:])
            projb = constp.tile([32, 1], F32)
            nc.sync.dma_start(projb[:], projb_d[:])
            ones_row = constp.tile([1, BC], MM_DT)
            nc.vector.memset(ones_row[:], 1.0)
            dec_in = constp.tile([CA, BC], MM_DT)
            nc.vector.memset(dec_in[C : C + 1, :], 1.0)

            def gru_step(wh, wi, bhn, x_ap, h_prev, gi_first):
                # PSUM accumulation groups must be sequential per bank (2KB
                # "zero region"): each region's [open ... close] matmuls stay
                # contiguous in PE program order.
                psum_rz = psump.tile([128, 512], F32, name="psum_rz")
                psum_n = psump.tile([128, 512], F32, name="psum_n")

                def rz_groups(ms):
                    for m in ms:
                        seg = psum_rz[:, m * 128 : (m + 1) * 128]
                        gi = (
                            wi[:, m * 128 : (m + 1) * 128], x_ap,
                        )
                        wh0 = (
                            wh[:, (m * 2) * 128 : (m * 2 + 1) * 128],
                            h_prev[:, 0:128],
                        )
                        wh1 = (
                            wh[:, (m * 2 + 1) * 128 : (m * 2 + 2) * 128],
                            h_prev[:, 128:256],
                        )
                        ops = [gi, wh0, wh1] if gi_first else [wh0, wh1, gi]
                        for i, (lhsT, rhs) in enumerate(ops):
                            nc.tensor.matmul(
                                seg, lhsT, rhs, start=(i == 0), stop=(i == 2)
                            )

                def ghn_groups():
                    for cc in range(2):
                        seg = psum_n[:, cc * 128 : (cc + 1) * 128]
                        m = 4 + cc
                        nc.tensor.matmul(
                            seg, bhn[:, cc * 128 : (cc + 1) * 128], ones_row[:],
                            start=True, stop=False,
                        )
                        nc.tensor.matmul(
                            seg, wh[:, (m * 2) * 128 : (m * 2 + 1) * 128],
                            h_prev[:, 0:128], start=False, stop=False,
                        )
                        nc.tensor.matmul(
                            seg, wh[:, (m * 2 + 1) * 128 : (m * 2 + 2) * 128],
                            h_prev[:, 128:256], start=False, stop=True,
                        )

                def gin_groups():
                    for cc in range(2):
                        nc.tensor.matmul(
                            psum_n[:, 256 + cc * 128 : 256 + (cc + 1) * 128],
                            wi[:, (4 + cc) * 128 : (5 + cc) * 128], x_ap,
                            start=True, stop=True,
                        )

                # PE order: r regions first (unblocks sig_r), then ghn (t1's
                # other input), then z regions, then gin. Decoder puts ghn
                # first so pred-independent work hides the pred->gi latency.
                if gi_first:
                    rz_groups([0, 1]); ghn_groups(); rz_groups([2, 3]); gin_groups()
                else:
                    ghn_groups(); rz_groups([0, 1]); rz_groups([2, 3]); gin_groups()

                rz = workp.tile([128, 512], GATE_DT, name="rz")
                r_ap, z_ap = rz[:, 0:256], rz[:, 256:512]
                t1 = workp.tile([128, 256], GATE_DT, name="t1")
                if SPLIT_SIG:
                    nc.scalar.activation(r_ap, psum_rz[:, 0:256], AF.Sigmoid)
                    nc.vector.tensor_tensor(t1[:], psum_n[:, 0:256], r_ap, OP.mult)
                    nc.scalar.activation(z_ap, psum_rz[:, 256:512], AF.Sigmoid)
                else:
                    nc.scalar.activation(rz[:], psum_rz[:], AF.Sigmoid)
                    nc.vector.tensor_tensor(t1[:], psum_n[:, 0:256], r_ap, OP.mult)
                q = workp.tile([128, 256], GATE_DT, name="q")
                nc.vector.tensor_tensor(q[:], t1[:], psum_n[:, 256:512], OP.add)
                # zb = 1 - z and c1 = z*h run during the tanh window
                eng = nc.gpsimd if GP_OFFLOAD else nc.vector
                zb = workp.tile([128, 256], GATE_DT, name="zb")
                eng.tensor_scalar(zb[:], z_ap, -1.0, 1.0, OP.mult, OP.add)
                c1 = workp.tile([128, 256], GATE_DT, name="c1")
                eng.tensor_tensor(c1[:], z_ap, h_prev[:], OP.mult)
                n_t = workp.tile([128, 256], GATE_DT, name="n_t")
                nc.scalar.activation(n_t[:], q[:], AF.Tanh)
                u_t = workp.tile([128, 256], GATE_DT, name="u_t")
                nc.vector.tensor_tensor(u_t[:], zb[:], n_t[:], OP.mult)
                h_new = statep.tile([128, 256], GATE_DT, name="h")
                nc.vector.tensor_add(h_new[:], c1[:], u_t[:])
                return h_new

            def body():
                nc.vector.memset(dec_in[0:C, :], 0.0)
                h = statep.tile([128, 256], GATE_DT, name="h")
                nc.vector.memset(h[:], 0.0)

                # ---- encoder ----
                for blk in range(nblk):
                    xb = xp.tile([CA, xblk * BC], MM_DT, name="xb")
                    nc.sync.dma_start(xb[:], xd[blk])
                    for j in range(xblk):
                        if blk * xblk + j >= t_steps:
                            break
                        h = gru_step(
                            whh_e, wih_e, bhn_e, xb[:, j * BC : (j + 1) * BC], h, True
                        )

                # ---- decoder ----
                for t in range(t_steps):
                    h = gru_step(whh_d, wih_d, bhn_d, dec_in[:], h, False)
                    psum_p = psump.tile([32, BC], F32, name="psum_p", bufs=2)
                    nc.tensor.matmul(
                        psum_p[:], projT[:, 0:32], h[:, 0:128], start=True, stop=False
                    )
                    nc.tensor.matmul(
                        psum_p[:], projT[:, 32:64], h[:, 128:256],
                        start=False, stop=True,
                    )
                    # on-chain: feed pred straight into dec_in (fp16);
                    # off-chain: fp32 copy for the y output DMA
                    nc.vector.tensor_scalar_add(dec_in[0:C, :], psum_p[:], projb[:])
                    pred = workp.tile([32, BC], F32, name="pred")
                    nc.vector.tensor_scalar_add(pred[:], psum_p[:], projb[:])
                    nc.sync.dma_start(yd[t], pred[:])

            if reps == 1:
                body()
            else:
                with tc.For_i(0, reps):
                    body()

    nc.compile()
    return nc




def build_2g(t_steps=T, reps=1):
    """Two-group (batch 64+64) software-pipelined variant: two independent
    recurrence chains per core fill each other's cross-engine latency."""
    G = BC // 2  # 64
    nblk = (t_steps + XBLK - 1) // XBLK
    assert t_steps % XBLK == 0 or t_steps < XBLK
    xblk = min(XBLK, t_steps)
    nc = bacc.Bacc("TRN2", num_devices=NCORES)

    xd = nc.dram_tensor("x_t", [nblk, CA, xblk * BC], MM_DT, kind="ExternalInput").ap()
    whh_e_d = nc.dram_tensor("whh_e", [128, 12 * 128], MM_DT, kind="ExternalInput").ap()
    whh_d_d = nc.dram_tensor("whh_d", [128, 12 * 128], MM_DT, kind="ExternalInput").ap()
    wih_e_d = nc.dram_tensor("wih_e", [CA, 768], MM_DT, kind="ExternalInput").ap()
    wih_d_d = nc.dram_tensor("wih_d", [CA, 768], MM_DT, kind="ExternalInput").ap()
    bhn_e_d = nc.dram_tensor("bhn_e", [1, 256], MM_DT, kind="ExternalInput").ap()
    bhn_d_d = nc.dram_tensor("bhn_d", [1, 256], MM_DT, kind="ExternalInput").ap()
    projT_d = nc.dram_tensor("projT", [128, 64], MM_DT, kind="ExternalInput").ap()
    projb_d = nc.dram_tensor("projb", [32, 1], F32, kind="ExternalInput").ap()
    yd = nc.dram_tensor("y_t", [t_steps, C, BC], F32, kind="ExternalOutput").ap()

    with tile.TileContext(nc) as tc:
        with (
            tc.tile_pool(name="const", bufs=1) as constp,
            tc.tile_pool(name="xp", bufs=2) as xp,
            tc.tile_pool(name="state", bufs=2) as statep,
            tc.tile_pool(name="work", bufs=2) as workp,
            tc.tile_pool(name="psum", bufs=2, space="PSUM") as psump,
        ):
            whh_e = constp.tile([128, 1536], MM_DT)
            nc.sync.dma_start(whh_e[:], whh_e_d[:])
            whh_d = constp.tile([128, 1536], MM_DT)
            nc.sync.dma_start(whh_d[:], whh_d_d[:])
            wih_e = constp.tile([CA, 768], MM_DT)
            nc.sync.dma_start(wih_e[:], wih_e_d[:])
            wih_d = constp.tile([CA, 768], MM_DT)
            nc.sync.dma_start(wih_d[:], wih_d_d[:])
            bhn_e = constp.tile([1, 256], MM_DT)
            nc.sync.dma_start(bhn_e[:], bhn_e_d[:])
            bhn_d = constp.tile([1, 256], MM_DT)
            nc.sync.dma_start(bhn_d[:], bhn_d_d[:])
            projT = constp.tile([128, 64], MM_DT)
            nc.sync.dma_start(projT[:], projT_d[:])
            projb = constp.tile([32, 1], F32)
            nc.sync.dma_start(projb[:], projb_d[:])
            ones_row = constp.tile([1, G], MM_DT)
            nc.vector.memset(ones_row[:], 1.0)
            dec_in = constp.tile([CA, BC], MM_DT)
            nc.vector.memset(dec_in[C : C + 1, :], 1.0)

            def emit_pe(wh, wi, bhn, x_ap, h_prev, gi_first, psum_rz, psum_n):
                # h_prev: [128, 2*G]; x_ap: [CA, G]
                def rz_group(m):
                    seg = psum_rz[:, m * G : (m + 1) * G]
                    gi = (wi[:, m * 128 : (m + 1) * 128], x_ap)
                    wh0 = (wh[:, (m * 2) * 128 : (m * 2 + 1) * 128], h_prev[:, 0:G])
                    wh1 = (
                        wh[:, (m * 2 + 1) * 128 : (m * 2 + 2) * 128],
                        h_prev[:, G : 2 * G],
                    )
                    ops = [gi, wh0, wh1] if gi_first else [wh0, wh1, gi]
                    for i, (lhsT, rhs) in enumerate(ops):
                        nc.tensor.matmul(seg, lhsT, rhs, start=(i == 0), stop=(i == 2))

                def ghn_group(cc):
                    seg = psum_n[:, cc * G : (cc + 1) * G]
                    m = 4 + cc
                    nc.tensor.matmul(
                        seg, bhn[:, cc * 128 : (cc + 1) * 128], ones_row[:],
                        start=True, stop=False,
                    )
                    nc.tensor.matmul(
                        seg, wh[:, (m * 2) * 128 : (m * 2 + 1) * 128],
                        h_prev[:, 0:G], start=False, stop=False,
                    )
                    nc.tensor.matmul(
                        seg, wh[:, (m * 2 + 1) * 128 : (m * 2 + 2) * 128],
                        h_prev[:, G : 2 * G], start=False, stop=True,
                    )

                def gin_group(cc):
                    nc.tensor.matmul(
                        psum_n[:, 2 * G + cc * G : 2 * G + (cc + 1) * G],
                        wi[:, (4 + cc) * 128 : (5 + cc) * 128], x_ap,
                        start=True, stop=True,
                    )

                if gi_first:
                    for m in (0, 1):
                        rz_group(m)
                    ghn_group(0); ghn_group(1)
                    for m in (2, 3):
                        rz_group(m)
                    gin_group(0); gin_group(1)
                else:
                    ghn_group(0); ghn_group(1)
                    for m in (0, 1, 2, 3):
                        rz_group(m)
                    gin_group(0); gin_group(1)

            def gates_front(psum_rz, psum_n, g):
                rz = workp.tile([128, 2 * 256 // 2], GATE_DT, name=f"rz{g}")
                nc.scalar.activation(rz[:], psum_rz[:], AF.Sigmoid)
                return rz

            def gates_mid(psum_n, rz, g):
                t1 = workp.tile([128, 2 * G], GATE_DT, name=f"t1{g}")
                nc.vector.tensor_tensor(
                    t1[:], psum_n[:, 0 : 2 * G], rz[:, 0 : 2 * G], OP.mult
                )
                q = workp.tile([128, 2 * G], GATE_DT, name=f"q{g}")
                nc.vector.tensor_tensor(
                    q[:], t1[:], psum_n[:, 2 * G : 4 * G], OP.add
                )
                return q

            def gates_tanh(q, g):
                n_t = workp.tile([128, 2 * G], GATE_DT, name=f"n{g}")
                nc.scalar.activation(n_t[:], q[:], AF.Tanh)
                return n_t

            def gates_tail(h_prev, n_t, rz, g):
                d_t = workp.tile([128, 2 * G], GATE_DT, name=f"d{g}")
                nc.vector.tensor_tensor(d_t[:], h_prev[:], n_t[:], OP.subtract)
                e_t = workp.tile([128, 2 * G], GATE_DT, name=f"e{g}")
                nc.vector.tensor_tensor(e_t[:], d_t[:], rz[:, 2 * G : 4 * G], OP.mult)
                h_new = statep.tile([128, 2 * G], GATE_DT, name=f"h{g}")
                nc.vector.tensor_add(h_new[:], e_t[:], n_t[:])
                return h_new

            def step_pair(wh, wi, bhn, x_aps, hs, gi_first):
                prz = [
                    psump.tile([128, 4 * G], F32, name=f"psum_rz{g}")
                    for g in range(2)
                ]
                pn = [
                    psump.tile([128, 6 * G], F32, name=f"psum_n{g}")
                    for g in range(2)
                ]
                for g in range(2):
                    emit_pe(wh, wi, bhn, x_aps[g], hs[g], gi_first, prz[g], pn[g])
                rzA = gates_front(prz[0], pn[0], 0)
                qA = gates_mid(pn[0], rzA, 0)
                rzB = gates_front(prz[1], pn[1], 1)
                nA = gates_tanh(qA, 0)
                qB = gates_mid(pn[1], rzB, 1)
                hA = gates_tail(hs[0], nA, rzA, 0)
                nB = gates_tanh(qB, 1)
                hB = gates_tail(hs[1], nB, rzB, 1)
                return [hA, hB], pn

            hs = None

            def body():
                nonlocal hs
                nc.vector.memset(dec_in[0:C, :], 0.0)
                h0 = statep.tile([128, 2 * G], GATE_DT, name="h0")
                nc.vector.memset(h0[:], 0.0)
                h1 = statep.tile([128, 2 * G], GATE_DT, name="h1")
                nc.vector.memset(h1[:], 0.0)
                hs = [h0, h1]

                for blk in range(nblk):
                    xb = xp.tile([CA, xblk * BC], MM_DT, name="xb")
                    nc.sync.dma_start(xb[:], xd[blk])
                    for j in range(xblk):
                        if blk * xblk + j >= t_steps:
                            break
                        x_aps = [
                            xb[:, j * BC + g * G : j * BC + (g + 1) * G]
                            for g in range(2)
                        ]
                        hs, _ = step_pair(whh_e, wih_e, bhn_e, x_aps, hs, True)

                for t in range(t_steps):
                    x_aps = [dec_in[:, g * G : (g + 1) * G] for g in range(2)]
                    hs, pn = step_pair(whh_d, wih_d, bhn_d, x_aps, hs, False)
                    for g in range(2):
                        pred_seg = pn[g][0:32, 4 * G : 4 * G + G]
                        nc.tensor.matmul(
                            pred_seg, projT[:, 0:32], hs[g][:, 0:G],
                            start=True, stop=False,
                        )
                        nc.tensor.matmul(
                            pred_seg, projT[:, 32:64], hs[g][:, G : 2 * G],
                            start=False, stop=True,
                        )
                        nc.vector.tensor_scalar_add(
                            dec_in[0:C, g * G : (g + 1) * G], pred_seg, projb[:]
                        )
                        pred = workp.tile([32, G], F32, name=f"pred{g}")
                        nc.vector.tensor_scalar_add(pred[:], pred_seg, projb[:])
                        nc.sync.dma_start(yd[t][:, g * G : (g + 1) * G], pred[:])

            if reps == 1:
                body()
            else:
                with tc.For_i(0, reps):
                    body()

    nc.compile()
    return nc


def prep_inputs(x, enc_Wih, enc_Whh, enc_bih, enc_bhh,
                dec_Wih, dec_Whh, dec_bih, dec_bhh, proj_W, proj_b,
                t_steps=T):
    """Host-side shard + relayout. Returns in_maps (list of 8 dicts)."""
    nblk = (t_steps + XBLK - 1) // XBLK
    xblk = min(XBLK, t_steps)

    def whh_pack(W):
        WT = np.ascontiguousarray(W.T)  # [H, 3H]
        out = np.empty((128, 12 * 128), np.float32)
        for m in range(6):
            for k in range(2):
                out[:, (m * 2 + k) * 128 : (m * 2 + k + 1) * 128] = WT[
                    k * 128 : (k + 1) * 128, m * 128 : (m + 1) * 128
                ]
        return out.astype(NP_MM)

    def wih_pack(Wih, bih, bhh):
        fold = bih.astype(np.float64)
        fold[: 2 * H] += bhh[: 2 * H]
        Wa = np.concatenate([Wih.astype(np.float64), fold[:, None]], axis=1)
        return np.ascontiguousarray(Wa.T).astype(NP_MM)  # [CA, 3H]

    def proj_pack(W):
        out = np.empty((128, 64), np.float32)
        for k in range(2):
            out[:, 32 * k : 32 * k + 32] = W[:, k * 128 : (k + 1) * 128].T
        return out.astype(NP_MM)

    shared = {
        "whh_e": whh_pack(enc_Whh),
        "whh_d": whh_pack(dec_Whh),
        "wih_e": wih_pack(enc_Wih, enc_bih, enc_bhh),
        "wih_d": wih_pack(dec_Wih, dec_bih, dec_bhh),
        "bhn_e": np.ascontiguousarray(enc_bhh[2 * H :][None, :]).astype(NP_MM),
        "bhn_d": np.ascontiguousarray(dec_bhh[2 * H :][None, :]).astype(NP_MM),
        "projT": proj_pack(proj_W),
        "projb": np.ascontiguousarray(proj_b[:, None].astype(np.float32)),
    }
    in_maps = []
    for i in range(NCORES):
        xc = x[i * BC : (i + 1) * BC, :t_steps]  # [BC, t, C]
        xt = np.concatenate(
            [xc.transpose(1, 2, 0), np.ones((t_steps, 1, BC), np.float32)], axis=1
        )  # [t, CA, BC]
        xt = (
            xt.reshape(nblk, xblk, CA, BC)
            .transpose(0, 2, 1, 3)
            .reshape(nblk, CA, xblk * BC)
        )
        m = dict(shared)
        m["x_t"] = np.ascontiguousarray(xt).astype(NP_MM)
        in_maps.append(m)
    return in_maps


_BUILT = {}


BUILDER = build  # build_2g (2-chain pipeline) measured slower: 6.14 vs 5.92 ms


def run(inputs, t_steps=T, trace=False):
    if t_steps not in _BUILT:
        _BUILT[t_steps] = BUILDER(t_steps)
    nc = _BUILT[t_steps]
    in_maps = prep_inputs(**inputs, t_steps=t_steps)
    res = run_bass_kernel_spmd(nc, in_maps, core_ids=list(range(NCORES)), trace=trace)
    outs = []
    for i in range(NCORES):
        yt = res.results[i]["y_t"]  # [t, C, BC]
        outs.append(np.ascontiguousarray(yt.transpose(2, 0, 1)))  # [BC, t, C]
    y = np.concatenate(outs, axis=0)
    return y, res


def kernel(**inputs):
    y, _ = run(inputs, t_steps=T, trace=False)
    return y

